# revision 35
# baseline (speedup 1.0000x reference)
"""Trainium2 Bass kernel for batched Gaussian log-density quadratic form.

Computes out = -einsum('nd,de,ne->n', Y, prec, Y) with Y = X - mean,
X: [65536, 256] f32, mean: [1, 256] f32, prec: [256, 256] f32.

Strategy (data-parallel over rows, 8 NeuronCores):
  Only the symmetric part S = (prec + prec^T)/2 contributes. The host
  eigendecomposes S = Q diag(lam) Q^T and uploads V = Q sqrt(|lam|) in
  bf16, columns arranged so chunk1 (k in [128,256)) holds 128 columns of
  the majority class of s_k = -sign(lam_k) and chunk0 the rest. Then

      out[n] = sum_k s_k * (Y V)[n,k]^2

  so the PSUM egress IS the squaring - no Z*Y elementwise product, no
  separate drain. Per 512-column window (z PSUM [128, 2 chunks, 512],
  4 bf16 matmuls of free=512 = 853ns on PE), two alternating styles:
    even: ACT square-drains both chunks (1038); DVE folds with ONE
          scalar_tensor_tensor wf = (za0*sv) +/- za1 (593)
    odd:  DVE drains chunk0 SIGNED via the fused (z0*sv)*z0
          scalar_tensor_tensor straight from PSUM (658); ACT square-
          drains only chunk1 (612); fold is a plain 2x tensor add/sub
          (327)
  Per pair: ACT 1650, DVE 1578, Pool 2x807 preduce, PE 1706 <- pacer.
  Steady state is PE-paced at ~853ns/window vs the 1038 ACT-drain wall
  of the multiply formulation. PSUM ring bufs=4 gives a 3-period
  recycle deadline so the drain chains never stall the matmuls.
  sv/the fold op are data-dependent (majority eigenvalue sign): the
  program is built per-sign at first call.
  Warmup matmuls (junk f32r tile memset on Pool, output into a corner
  of window 0's PSUM tile) finish the PE p-state ramp during the DMA
  fill. The last window is tapered into [256,128,128]-column sub-chains
  and output flushes go out in slices so the final DMA waits only on
  the last 128 columns.
"""

import numpy as np

N, D = 65536, 256
N_CORES = 8
NS = N // N_CORES  # 8192 rows per core
P = 128
SW = 1024  # DMA super-window (two compute windows)
NSW = NS // SW  # 8
W = 512  # compute window columns
NW = NS // W  # 16
N_WARM = 14
PREC_COLS = 4 * P  # 512 (V chunk block)
SV_COL = PREC_COLS  # 512
Y0_OFF = PREC_COLS + 2  # 514; super 0's d0 half rides in the preamble
PRE_COLS_TOTAL = Y0_OFF + SW  # 1538
# tail sub-ranges of the last window (col spans within [0,512))
TAIL_SUBS = [(0, 256), (256, 384), (384, 512)]

TRACE = False
LAST_EXEC_NS = None
LAST_RESULTS = None

_PROGRAMS = {}


def _build_program(op_is_add):
    import concourse.bass as bass
    import concourse.tile as tile
    from concourse import bacc, bass_isa, mybir
    from contextlib import ExitStack

    F32 = mybir.dt.float32
    F32R = mybir.dt.float32r
    BF16 = mybir.dt.bfloat16
    OP1 = mybir.AluOpType.add if op_is_add else mybir.AluOpType.subtract

    nc = bacc.Bacc("TRN2", target_bir_lowering=False, debug=False)
    yt_dram = nc.dram_tensor("yt", [NSW, P, 2, SW], BF16, kind="ExternalInput").ap()
    # packed preamble: [4x128 V chunks | sv (2 cols) | super 0's d0 half]
    pre_dram = nc.dram_tensor(
        "pre", [P, PRE_COLS_TOTAL], BF16, kind="ExternalInput"
    ).ap()
    out_dram = nc.dram_tensor("out", [1, NS], F32, kind="ExternalOutput").ap()

    with tile.TileContext(nc) as tc, ExitStack() as ctx:
        singles = ctx.enter_context(tc.tile_pool(name="singles", bufs=1))
        ytpool = ctx.enter_context(tc.tile_pool(name="ytpool", bufs=NSW))
        zbpool = ctx.enter_context(tc.tile_pool(name="zbpool", bufs=4))
        wfpool = ctx.enter_context(tc.tile_pool(name="wfpool", bufs=4))
        psum = ctx.enter_context(tc.tile_pool(name="psum", bufs=4, space="PSUM"))

        # f32 result staging: Pool's partition all-reduce writes window w's
        # 512 results (replicated across partitions; row 0 is DMA'd out)
        stage = singles.tile([P, NW, W], F32)

        warm = singles.tile([P, P], F32)
        nc.gpsimd.memset(warm, 0.25)
        warm_r = warm.bitcast(F32R)

        pre = singles.tile([P, PRE_COLS_TOTAL], BF16)
        nc.sync.dma_start(pre, pre_dram)

        zs = [None] * NW

        def get_z(w):
            if zs[w] is None:
                z = psum.tile([P, 2, W], F32, tag="z")
                zs[w] = z
            return zs[w]

        z0 = get_z(0)
        for _ in range(N_WARM):
            nc.tensor.matmul(
                z0[0:8, 0, 0:P],
                lhsT=warm_r[:, 0:8],
                rhs=warm_r,
                start=True,
                stop=True,
            )

        def vp(d, e):
            return pre[:, (2 * d + e) * P : (2 * d + e + 1) * P]

        sv = pre[:, SV_COL : SV_COL + 1]

        yts = [None] * NSW

        def issue_dma(s):
            yt = ytpool.tile([P, 2, SW], BF16, tag="yt")
            if s == 0:
                # d0 lives in the pre tile; only d1 arrives here
                nc.sync.dma_start(yt[:, 1, :], yt_dram[0][:, 1, :])
            else:
                nc.sync.dma_start(yt, yt_dram[s])
            yts[s] = yt

        def yrhs(w, d, lo, hi):
            s, c0 = w // 2, (w % 2) * W
            if s == 0 and d == 0:
                return pre[:, Y0_OFF + c0 + lo : Y0_OFF + c0 + hi]
            return yts[s][:, d, c0 + lo : c0 + hi]

        def issue_mm(w, lo=0, hi=W):
            z = get_z(w)
            if w // 2 == 0:
                # d-major: d0 matmuls run off the pre tile while super 0's
                # d1 half is still in flight
                for e in range(2):
                    nc.tensor.matmul(
                        z[:, e, lo:hi], lhsT=vp(0, e), rhs=yrhs(w, 0, lo, hi),
                        start=True, stop=False,
                    )
                for e in range(2):
                    nc.tensor.matmul(
                        z[:, e, lo:hi], lhsT=vp(1, e), rhs=yrhs(w, 1, lo, hi),
                        start=False, stop=True,
                    )
            else:
                for e in range(2):
                    for d in range(2):
                        nc.tensor.matmul(
                            z[:, e, lo:hi], lhsT=vp(d, e), rhs=yrhs(w, d, lo, hi),
                            start=(d == 0), stop=(d == 1),
                        )

        wf_pairs = {}

        def issue_post_even(w):
            # style A: ACT square-drains both chunks; DVE folds via stt.
            # The fold lands in slot 0 of a pair-shared wf tile; the paired
            # odd window issues one [128,1024] partition reduce for both
            # (759ns/win amortized vs 806+hops single - Pool was the only
            # engine running over the 853 PE pace).
            z = zs[w]
            za = zbpool.tile([P, 2, W], BF16, tag="za")
            wfp = wfpool.tile([P, 2, W], BF16, tag="wfp")
            wf_pairs[w] = wfp
            nc.scalar.square(za, z)
            nc.vector.scalar_tensor_tensor(
                wfp[:, 0], za[:, 0], sv, za[:, 1], mybir.AluOpType.mult, OP1
            )

        def issue_post_odd(w):
            # style B: DVE signed-square-drains chunk0 in one fused stt
            # straight from PSUM; ACT square-drains chunk1; 2x-mode fold
            z = zs[w]
            t0 = zbpool.tile([P, W], BF16, tag="t0")
            za1 = zbpool.tile([P, W], BF16, tag="za1")
            wfp = wf_pairs[w - 1]
            nc.vector.scalar_tensor_tensor(
                t0, z[:, 0], sv, z[:, 0],
                mybir.AluOpType.mult, mybir.AluOpType.mult,
            )
            nc.scalar.square(za1, z[:, 1])
            if op_is_add:
                nc.vector.tensor_add(wfp[:, 1], t0, za1)
            else:
                nc.vector.tensor_sub(wfp[:, 1], t0, za1)
            nc.gpsimd.partition_all_reduce(
                stage[:, w - 1 : w + 1], wfp, P, bass_isa.ReduceOp.add
            )

        def issue_sub_post_a(w, lo, hi):
            # small A-style sub-chain on [lo,hi)
            z = zs[w]
            L = hi - lo
            za = zbpool.tile([P, 2, L], BF16, tag=f"zat{w}_{lo}")
            wf = wfpool.tile([P, L], BF16, tag=f"wft{w}_{lo}")
            nc.scalar.square(za, z[:, :, lo:hi])
            nc.vector.scalar_tensor_tensor(
                wf, za[:, 0], sv, za[:, 1], mybir.AluOpType.mult, OP1
            )
            nc.gpsimd.partition_all_reduce(
                stage[:, w, lo:hi], wf, P, bass_isa.ReduceOp.add
            )

        def issue_sub_post_b(w, lo, hi):
            # small B-style sub-chain: DVE signed chunk0 drain, ACT chunk1
            z = zs[w]
            L = hi - lo
            t0 = zbpool.tile([P, L], BF16, tag=f"t0t{w}_{lo}")
            za1 = zbpool.tile([P, L], BF16, tag=f"za1t{w}_{lo}")
            wf = wfpool.tile([P, L], BF16, tag=f"wfbt{w}_{lo}")
            nc.vector.scalar_tensor_tensor(
                t0, z[:, 0, lo:hi], sv, z[:, 0, lo:hi],
                mybir.AluOpType.mult, mybir.AluOpType.mult,
            )
            nc.scalar.square(za1, z[:, 1, lo:hi])
            if op_is_add:
                nc.vector.tensor_add(wf, t0, za1)
            else:
                nc.vector.tensor_sub(wf, t0, za1)
            nc.gpsimd.partition_all_reduce(
                stage[:, w, lo:hi], wf, P, bass_isa.ReduceOp.add
            )

        for s in range(NSW):
            issue_dma(s)

        issue_mm(0)
        issue_post_even(0)

        for w in range(1, NW - 1):
            issue_mm(w)
            if w % 2 == 0:
                issue_post_even(w)
            else:
                issue_post_odd(w)
            if w == 8:
                nc.sync.dma_start(out_dram[:, 0 : 8 * W], stage[0:1, 0:8])
            if w == 14:
                nc.sync.dma_start(out_dram[:, 8 * W : 14 * W], stage[0:1, 8:14])

        # w14 has no pair partner: single reduce for its fold slot
        nc.gpsimd.partition_all_reduce(
            stage[:, 14], wf_pairs[14][:, 0], P, bass_isa.ReduceOp.add
        )

        # tapered last window, engines interleaved across sub-chains
        w = NW - 1
        for lo, hi in TAIL_SUBS:
            issue_mm(w, lo, hi)
        for i, (lo, hi) in enumerate(TAIL_SUBS):
            issue_sub_post_a(w, lo, hi)
            if hi == 384:
                nc.sync.dma_start(
                    out_dram[:, 14 * W : 15 * W], stage[0:1, 14]
                )
                nc.sync.dma_start(
                    out_dram[:, 15 * W : 15 * W + 384], stage[0:1, 15, 0:384]
                )

        nc.sync.dma_start(
            out_dram[:, 15 * W + 384 : NS], stage[0:1, 15, 384:512]
        )

    nc.compile()

    return nc


def _get_program(op_is_add):
    key = bool(op_is_add)
    if key not in _PROGRAMS:
        _PROGRAMS[key] = _build_program(key)
    return _PROGRAMS[key]


def _host_prep(X, mean, prec):
    import ml_dtypes

    bf16 = ml_dtypes.bfloat16
    Xf = np.asarray(X, dtype=np.float32)
    m = np.asarray(mean, dtype=np.float32).reshape(1, D)
    Y = (Xf - m).astype(bf16)  # [N, 256]

    S = np.asarray(prec, dtype=np.float64)
    S = (S + S.T) * 0.5
    lam, Q = np.linalg.eigh(S)
    shat = -np.sign(lam)
    shat[shat == 0] = 1.0
    maj = 1.0 if (shat > 0).sum() >= P else -1.0
    majcols = np.where(shat == maj)[0]
    mincols = np.where(shat != maj)[0]
    chunk1 = majcols[:P]
    chunk0 = np.concatenate([majcols[P:], mincols])
    order = np.concatenate([chunk0, chunk1])
    V = (Q * np.sqrt(np.abs(lam)))[:, order].astype(bf16)  # [256, 256]
    sv0 = shat[chunk0].astype(bf16)  # [128]
    op_is_add = maj > 0

    pre_base = np.zeros((P, PRE_COLS_TOTAL), dtype=bf16)
    # pre[p, (2d+e)*128 + c] = V[128d + p, 128e + c]
    pre_base[:, :PREC_COLS] = (
        V.reshape(2, P, 2, P).transpose(1, 0, 2, 3).reshape(P, PREC_COLS)
    )
    pre_base[:, SV_COL] = sv0

    in_maps = []
    for i in range(N_CORES):
        Yc = Y[i * NS : (i + 1) * NS]  # [8192, 256]
        # yt[s, p, d, j] = Yc[1024 s + j, 128 d + p]
        yt = np.ascontiguousarray(
            Yc.reshape(NSW, SW, 2, P).transpose(0, 3, 2, 1)
        )
        pre_host = pre_base.copy()
        pre_host[:, Y0_OFF:] = yt[0, :, 0, :]
        in_maps.append({"yt": yt, "pre": pre_host})
    return in_maps, op_is_add


def kernel(X, mean, prec):
    global LAST_EXEC_NS, LAST_RESULTS
    from concourse.bass_utils import run_bass_kernel_spmd

    in_maps, op_is_add = _host_prep(X, mean, prec)
    nc = _get_program(op_is_add)
    res = run_bass_kernel_spmd(
        nc, in_maps, core_ids=list(range(N_CORES)), trace=TRACE
    )
    LAST_RESULTS = res
    LAST_EXEC_NS = res.exec_time_ns
    out = np.concatenate(
        [res.results[i]["out"].reshape(NS) for i in range(N_CORES)]
    )
    return out.astype(np.float32)


# revision 36
# speedup vs baseline: 1.0223x; 1.0223x over previous
"""Trainium2 Bass kernel for batched Gaussian log-density quadratic form.

Computes out = -einsum('nd,de,ne->n', Y, prec, Y) with Y = X - mean,
X: [65536, 256] f32, mean: [1, 256] f32, prec: [256, 256] f32.

Strategy (data-parallel over rows, 8 NeuronCores):
  Only the symmetric part S = (prec + prec^T)/2 contributes. The host
  eigendecomposes S = Q diag(lam) Q^T and uploads V = Q sqrt(|lam|) in
  bf16, columns arranged so chunk1 (k in [128,256)) holds 128 columns of
  the majority class of s_k = -sign(lam_k) and chunk0 the rest. Then

      out[n] = sum_k s_k * (Y V)[n,k]^2

  so the PSUM egress IS the squaring - no Z*Y elementwise product, no
  separate drain. Per 512-column window (z PSUM [128, 2 chunks, 512],
  4 bf16 matmuls of free=512 = 853ns on PE), two alternating styles:
    even: ACT square-drains both chunks (1038); DVE folds with ONE
          scalar_tensor_tensor wf = (za0*sv) +/- za1 (593)
    odd:  DVE drains chunk0 SIGNED via the fused (z0*sv)*z0
          scalar_tensor_tensor straight from PSUM (658); ACT square-
          drains only chunk1 (612); fold is a plain 2x tensor add/sub
          (327)
  Per pair: ACT 1650, DVE 1578, Pool 2x807 preduce, PE 1706 <- pacer.
  Steady state is PE-paced at ~853ns/window vs the 1038 ACT-drain wall
  of the multiply formulation. PSUM ring bufs=4 gives a 3-period
  recycle deadline so the drain chains never stall the matmuls.
  sv/the fold op are data-dependent (majority eigenvalue sign): the
  program is built per-sign at first call.
  Warmup matmuls (junk f32r tile memset on Pool, output into a corner
  of window 0's PSUM tile) finish the PE p-state ramp during the DMA
  fill. The last window is tapered into [256,128,128]-column sub-chains
  and output flushes go out in slices so the final DMA waits only on
  the last 128 columns.
"""

import numpy as np

N, D = 65536, 256
N_CORES = 8
NS = N // N_CORES  # 8192 rows per core
P = 128
SW = 1024  # DMA super-window (two compute windows)
NSW = NS // SW  # 8
W = 512  # compute window columns
NW = NS // W  # 16
N_WARM = 14
PREC_COLS = 4 * P  # 512 (V chunk block)
SV_COL = PREC_COLS  # 512
Y0_OFF = PREC_COLS + 2  # 514; super 0's d0 half rides in the preamble
PRE_COLS_TOTAL = Y0_OFF + SW  # 1538
# tail sub-ranges of the last window (col spans within [0,512))
TAIL_SUBS = [(0, 256), (256, 384), (384, 512)]

TRACE = False
LAST_EXEC_NS = None
LAST_RESULTS = None

_PROGRAMS = {}


def _build_program(op_is_add):
    import concourse.bass as bass
    import concourse.tile as tile
    from concourse import bacc, bass_isa, mybir
    from contextlib import ExitStack

    F32 = mybir.dt.float32
    F32R = mybir.dt.float32r
    BF16 = mybir.dt.bfloat16
    OP1 = mybir.AluOpType.add if op_is_add else mybir.AluOpType.subtract

    nc = bacc.Bacc("TRN2", target_bir_lowering=False, debug=False)
    yt_dram = nc.dram_tensor("yt", [NSW, P, 2, SW], BF16, kind="ExternalInput").ap()
    # packed preamble: [4x128 V chunks | sv (2 cols) | super 0's d0 half]
    pre_dram = nc.dram_tensor(
        "pre", [P, PRE_COLS_TOTAL], BF16, kind="ExternalInput"
    ).ap()
    out_dram = nc.dram_tensor("out", [1, NS], F32, kind="ExternalOutput").ap()

    with tile.TileContext(nc) as tc, ExitStack() as ctx:
        singles = ctx.enter_context(tc.tile_pool(name="singles", bufs=1))
        ytpool = ctx.enter_context(tc.tile_pool(name="ytpool", bufs=NSW))
        zbpool = ctx.enter_context(tc.tile_pool(name="zbpool", bufs=4))
        wfpool = ctx.enter_context(tc.tile_pool(name="wfpool", bufs=4))
        psum = ctx.enter_context(tc.tile_pool(name="psum", bufs=4, space="PSUM"))

        # f32 result staging: Pool's partition all-reduce writes window w's
        # 512 results (replicated across partitions; row 0 is DMA'd out)
        stage = singles.tile([P, NW, W], F32)

        warm = singles.tile([P, P], F32)
        nc.gpsimd.memset(warm, 0.25)
        warm_r = warm.bitcast(F32R)

        pre = singles.tile([P, PRE_COLS_TOTAL], BF16)
        nc.sync.dma_start(pre, pre_dram)

        zs = [None] * NW

        def get_z(w):
            if zs[w] is None:
                z = psum.tile([P, 2, W], F32, tag="z")
                zs[w] = z
            return zs[w]

        z0 = get_z(0)
        for _ in range(N_WARM):
            nc.tensor.matmul(
                z0[0:8, 0, 0:P],
                lhsT=warm_r[:, 0:8],
                rhs=warm_r,
                start=True,
                stop=True,
            )

        def vp(d, e):
            return pre[:, (2 * d + e) * P : (2 * d + e + 1) * P]

        sv = pre[:, SV_COL : SV_COL + 1]

        yts = [None] * NSW

        def issue_dma(s):
            yt = ytpool.tile([P, 2, SW], BF16, tag="yt")
            if s == 0:
                # d0 lives in the pre tile; only d1 arrives here
                nc.sync.dma_start(yt[:, 1, :], yt_dram[0][:, 1, :])
            else:
                nc.sync.dma_start(yt, yt_dram[s])
            yts[s] = yt

        def yrhs(w, d, lo, hi):
            s, c0 = w // 2, (w % 2) * W
            if s == 0 and d == 0:
                return pre[:, Y0_OFF + c0 + lo : Y0_OFF + c0 + hi]
            return yts[s][:, d, c0 + lo : c0 + hi]

        def issue_mm(w, lo=0, hi=W):
            z = get_z(w)
            if w // 2 == 0:
                # d-major: d0 matmuls run off the pre tile while super 0's
                # d1 half is still in flight
                for e in range(2):
                    nc.tensor.matmul(
                        z[:, e, lo:hi], lhsT=vp(0, e), rhs=yrhs(w, 0, lo, hi),
                        start=True, stop=False,
                    )
                for e in range(2):
                    nc.tensor.matmul(
                        z[:, e, lo:hi], lhsT=vp(1, e), rhs=yrhs(w, 1, lo, hi),
                        start=False, stop=True,
                    )
            else:
                for e in range(2):
                    for d in range(2):
                        nc.tensor.matmul(
                            z[:, e, lo:hi], lhsT=vp(d, e), rhs=yrhs(w, d, lo, hi),
                            start=(d == 0), stop=(d == 1),
                        )

        def issue_post_even(w):
            # style A: ACT square-drains both chunks; DVE folds via stt
            z = zs[w]
            za = zbpool.tile([P, 2, W], BF16, tag="za")
            wf = wfpool.tile([P, W], BF16, tag="wf")
            nc.scalar.square(za, z)
            nc.vector.scalar_tensor_tensor(
                wf, za[:, 0], sv, za[:, 1], mybir.AluOpType.mult, OP1
            )
            nc.gpsimd.partition_all_reduce(
                stage[:, w], wf, P, bass_isa.ReduceOp.add
            )

        def issue_post_odd(w):
            # style B: DVE signed-square-drains chunk0 in one fused stt
            # straight from PSUM; ACT square-drains chunk1; 2x-mode fold
            z = zs[w]
            t0 = zbpool.tile([P, W], BF16, tag="t0")
            za1 = zbpool.tile([P, W], BF16, tag="za1")
            wf = wfpool.tile([P, W], BF16, tag="wfo")
            nc.vector.scalar_tensor_tensor(
                t0, z[:, 0], sv, z[:, 0],
                mybir.AluOpType.mult, mybir.AluOpType.mult,
            )
            nc.scalar.square(za1, z[:, 1])
            if op_is_add:
                nc.vector.tensor_add(wf, t0, za1)
            else:
                nc.vector.tensor_sub(wf, t0, za1)
            nc.gpsimd.partition_all_reduce(
                stage[:, w], wf, P, bass_isa.ReduceOp.add
            )

        def issue_sub_post_a(w, lo, hi):
            # small A-style sub-chain on [lo,hi)
            z = zs[w]
            L = hi - lo
            za = zbpool.tile([P, 2, L], BF16, tag=f"zat{w}_{lo}")
            wf = wfpool.tile([P, L], BF16, tag=f"wft{w}_{lo}")
            nc.scalar.square(za, z[:, :, lo:hi])
            nc.vector.scalar_tensor_tensor(
                wf, za[:, 0], sv, za[:, 1], mybir.AluOpType.mult, OP1
            )
            nc.gpsimd.partition_all_reduce(
                stage[:, w, lo:hi], wf, P, bass_isa.ReduceOp.add
            )

        def issue_sub_post_b(w, lo, hi):
            # small B-style sub-chain: DVE signed chunk0 drain, ACT chunk1
            z = zs[w]
            L = hi - lo
            t0 = zbpool.tile([P, L], BF16, tag=f"t0t{w}_{lo}")
            za1 = zbpool.tile([P, L], BF16, tag=f"za1t{w}_{lo}")
            wf = wfpool.tile([P, L], BF16, tag=f"wfbt{w}_{lo}")
            nc.vector.scalar_tensor_tensor(
                t0, z[:, 0, lo:hi], sv, z[:, 0, lo:hi],
                mybir.AluOpType.mult, mybir.AluOpType.mult,
            )
            nc.scalar.square(za1, z[:, 1, lo:hi])
            if op_is_add:
                nc.vector.tensor_add(wf, t0, za1)
            else:
                nc.vector.tensor_sub(wf, t0, za1)
            nc.gpsimd.partition_all_reduce(
                stage[:, w, lo:hi], wf, P, bass_isa.ReduceOp.add
            )

        for s in range(NSW):
            issue_dma(s)

        issue_mm(0)
        issue_post_even(0)

        for w in range(1, NW - 1):
            issue_mm(w)
            if w % 2 == 0:
                issue_post_even(w)
            else:
                issue_post_odd(w)
            if w == 8:
                nc.sync.dma_start(out_dram[:, 0 : 8 * W], stage[0:1, 0:8])
            if w == 14:
                nc.sync.dma_start(out_dram[:, 8 * W : 14 * W], stage[0:1, 8:14])

        # tapered last window, engines interleaved across sub-chains
        w = NW - 1
        for lo, hi in TAIL_SUBS:
            issue_mm(w, lo, hi)
        for i, (lo, hi) in enumerate(TAIL_SUBS):
            issue_sub_post_a(w, lo, hi)
            if hi == 384:
                nc.sync.dma_start(
                    out_dram[:, 14 * W : 15 * W], stage[0:1, 14]
                )
                nc.sync.dma_start(
                    out_dram[:, 15 * W : 15 * W + 384], stage[0:1, 15, 0:384]
                )

        nc.sync.dma_start(
            out_dram[:, 15 * W + 384 : NS], stage[0:1, 15, 384:512]
        )

    nc.compile()

    return nc


def _get_program(op_is_add):
    key = bool(op_is_add)
    if key not in _PROGRAMS:
        _PROGRAMS[key] = _build_program(key)
    return _PROGRAMS[key]


def _host_prep(X, mean, prec):
    import ml_dtypes

    bf16 = ml_dtypes.bfloat16
    Xf = np.asarray(X, dtype=np.float32)
    m = np.asarray(mean, dtype=np.float32).reshape(1, D)
    Y = (Xf - m).astype(bf16)  # [N, 256]

    S = np.asarray(prec, dtype=np.float64)
    S = (S + S.T) * 0.5
    lam, Q = np.linalg.eigh(S)
    shat = -np.sign(lam)
    shat[shat == 0] = 1.0
    maj = 1.0 if (shat > 0).sum() >= P else -1.0
    majcols = np.where(shat == maj)[0]
    mincols = np.where(shat != maj)[0]
    chunk1 = majcols[:P]
    chunk0 = np.concatenate([majcols[P:], mincols])
    order = np.concatenate([chunk0, chunk1])
    V = (Q * np.sqrt(np.abs(lam)))[:, order].astype(bf16)  # [256, 256]
    sv0 = shat[chunk0].astype(bf16)  # [128]
    op_is_add = maj > 0

    pre_base = np.zeros((P, PRE_COLS_TOTAL), dtype=bf16)
    # pre[p, (2d+e)*128 + c] = V[128d + p, 128e + c]
    pre_base[:, :PREC_COLS] = (
        V.reshape(2, P, 2, P).transpose(1, 0, 2, 3).reshape(P, PREC_COLS)
    )
    pre_base[:, SV_COL] = sv0

    in_maps = []
    for i in range(N_CORES):
        Yc = Y[i * NS : (i + 1) * NS]  # [8192, 256]
        # yt[s, p, d, j] = Yc[1024 s + j, 128 d + p]
        yt = np.ascontiguousarray(
            Yc.reshape(NSW, SW, 2, P).transpose(0, 3, 2, 1)
        )
        pre_host = pre_base.copy()
        pre_host[:, Y0_OFF:] = yt[0, :, 0, :]
        in_maps.append({"yt": yt, "pre": pre_host})
    return in_maps, op_is_add


def kernel(X, mean, prec):
    global LAST_EXEC_NS, LAST_RESULTS
    from concourse.bass_utils import run_bass_kernel_spmd

    in_maps, op_is_add = _host_prep(X, mean, prec)
    nc = _get_program(op_is_add)
    res = run_bass_kernel_spmd(
        nc, in_maps, core_ids=list(range(N_CORES)), trace=TRACE
    )
    LAST_RESULTS = res
    LAST_EXEC_NS = res.exec_time_ns
    out = np.concatenate(
        [res.results[i]["out"].reshape(NS) for i in range(N_CORES)]
    )
    return out.astype(np.float32)
